# revision 2
# baseline (speedup 1.0000x reference)
"""BitLinear Trainium2 kernel v2: 8-core token-sharded, pipelined quant+GEMM.

Design vs v1 baseline:
- Single pass over x (token-block slabs), quant pipelined per 128-token
  block so the GEMM can start ~30us in instead of ~260us.
- Cross-partition reductions on GPSIMD (partition_all_reduce) instead of
  PE transpose chains; only one small PE transpose per token block.
- Weight-group chains: Abs+accum (beta) and Sign on ACT, sum (alpha)
  split DVE/GPSIMD, staged f32 single-buffer.
- GEMM: per (group, tblock): KP fp8 DoubleRow matmuls (256-contraction
  each, 2x rate) + (KT-2*KP) bf16 matmuls, one PSUM bank.
  KP=0 is exact-int bf16 everywhere (rel err ~1e-7); KP>0 trades
  bounded fp8 rounding error (measured off-line) for speed.
"""

from contextlib import ExitStack

import numpy as np

import concourse.bass as bass
import concourse.bacc as bacc
import concourse.mybir as mybir
import concourse.tile as tile
import concourse.bass_isa as bass_isa
from concourse.bass_utils import run_bass_kernel_spmd
from concourse.masks import make_identity

F32 = mybir.dt.float32
BF16 = mybir.dt.bfloat16
FP8 = mybir.dt.float8e4
ALU = mybir.AluOpType
ACTF = mybir.ActivationFunctionType
DR = mybir.MatmulPerfMode.DoubleRow

MAGIC = 1.5 * 2**23  # fp32 round-to-nearest-even via (x + M) - M

N_CORES = 8
B, TT, IN_F = 4, 2048, 4096
OUT_F = 4096
G = 8
QB = 128.0
CLIP_EPS = 1e-6
T = (B * TT) // N_CORES  # tokens per core = 1024
KT = IN_F // 128         # 32 k-tiles
TB = T // 128            # 8 token blocks
GS = OUT_F // G          # 512 outs per group

KP = 7  # fp8 DoubleRow k-pairs per block (0..KT//2); fp8 covers k-tiles 0..2*KP-1


def free_view(t, ap_dims):
    """AP with explicit free-dim (stride, n) list on tile t's buffer."""
    return bass.AP(tensor=t.tensor, offset=t.offset,
                   ap=[list(t.ap[0])] + [list(d) for d in ap_dims])


def build_nc(kp=KP, w_chunk=4):
    nc = bacc.Bacc("TRN2", target_bir_lowering=False)
    xTb = nc.dram_tensor("xTb", [TB, 128, KT, 128], F32, kind="ExternalInput")
    wTp = nc.dram_tensor("wTp", [G, 128, KT, GS], F32, kind="ExternalInput")
    bias_d = nc.dram_tensor("bias", [OUT_F], F32, kind="ExternalInput")
    y = nc.dram_tensor("y", [T, OUT_F], F32, kind="ExternalOutput")

    n_ch = (KT + w_chunk - 1) // w_chunk
    inv_gin = 1.0 / (GS * IN_F)
    kb0 = 2 * kp  # first bf16 k-tile

    with ExitStack() as ctx:
        tc = ctx.enter_context(tile.TileContext(nc))
        singles = ctx.enter_context(tc.tile_pool(name="singles", bufs=1))
        xsp = ctx.enter_context(tc.tile_pool(name="xs", bufs=2))
        qspool = ctx.enter_context(tc.tile_pool(name="qsp", bufs=2))
        smalls = ctx.enter_context(tc.tile_pool(name="smalls", bufs=3))
        wstage = ctx.enter_context(tc.tile_pool(name="wstage", bufs=1))
        wbtp = ctx.enter_context(tc.tile_pool(name="wbt", bufs=2))
        brepp = ctx.enter_context(tc.tile_pool(name="brep", bufs=2))
        junkp = ctx.enter_context(tc.tile_pool(name="junk", bufs=1))
        ypool = ctx.enter_context(tc.tile_pool(name="yout", bufs=3))
        psum = ctx.enter_context(tc.tile_pool(name="psum", bufs=5, space="PSUM"))
        ps_small = ctx.enter_context(
            tc.tile_pool(name="ps_small", bufs=1, space="PSUM"))

        ones_col = singles.tile([128, 1], F32)
        nc.vector.memset(ones_col, 1.0)
        ones_row = singles.tile([1, 128], F32)
        nc.vector.memset(ones_row, 1.0)
        gcol_all = singles.tile([128, TB], F32)   # per-token gamma, tok on part
        qTb = None
        qT8 = None
        if kb0 < KT:
            qTb = singles.tile([128, KT - kb0, T], BF16)
        if kp > 0:
            qT8 = singles.tile([128, 2 * kp, T], FP8)

        # ---------------- quant: one 128-token block ----------------
        def emit_quant(tb):
            xs = xsp.tile([128, KT, 128], F32, tag="xs")
            nc.gpsimd.dma_start(out=xs, in_=xTb[tb, :, :, :])
            # per-(k-partition, token) absmax over kt (strided inner reduce)
            m1 = qspool.tile([128, 128], F32, tag="m1")
            xs_sw = free_view(xs, [(1, 128), (128, KT)])  # [p, tok, kt]
            nc.vector.tensor_reduce(out=m1, in_=xs_sw,
                                    axis=mybir.AxisListType.X, op=ALU.max,
                                    apply_absolute_value=True)
            # all-partition max -> every partition holds per-token gamma
            qs = qspool.tile([128, 128], F32, tag="qs")
            nc.gpsimd.partition_all_reduce(
                out_ap=qs[:, :], in_ap=m1[:, :], channels=128,
                reduce_op=bass_isa.ReduceOp.max)
            nc.vector.tensor_scalar(out=qs, in0=qs, scalar1=CLIP_EPS,
                                    scalar2=1.0 / QB, op0=ALU.max,
                                    op1=ALU.mult)
            # gamma column for dequant: transpose, keep col 0
            # qs holds gamma/QB replicated on every partition; a small
            # SBUF->SBUF DMA gathers row 0 into a per-token column (no PE)
            nc.gpsimd.dma_start(out=gcol_all[:, tb:tb + 1], in_=qs[0:1, :])
            nc.vector.reciprocal(qs, qs)
            # x *= qs (broadcast over kt), then round into qT8/qTb
            qs_b = free_view(qs, [(0, KT), (1, 128)])
            nc.vector.tensor_tensor(out=xs, in0=xs, in1=qs_b, op=ALU.mult)
            ts = tb * 128
            if kp > 0:
                nc.vector.tensor_scalar(
                    out=qT8[:, :, ts:ts + 128], in0=xs[:, 0:2 * kp, :],
                    scalar1=MAGIC, scalar2=MAGIC, op0=ALU.add,
                    op1=ALU.subtract)
            if qTb is not None:
                nc.vector.tensor_scalar(
                    out=qTb[:, :, ts:ts + 128], in0=xs[:, kb0:KT, :],
                    scalar1=MAGIC, scalar2=MAGIC, op0=ALU.add,
                    op1=ALU.subtract)

        # ---------------- weight chain for one group ----------------
        KH = KT // 2

        def chain_a(g):
            st0 = wstage.tile([128, KH, GS], F32, tag="stage0")
            st1 = wstage.tile([128, KH, GS], F32, tag="stage1")
            wbt = wbtp.tile([128, KT, GS], FP8, tag="wbt")
            sums_w = smalls.tile([128, KT], F32, tag="sums_w")
            sums_a = smalls.tile([128, n_ch], F32, tag="sums_a")
            for j, kc in enumerate(range(0, KT, w_chunk)):
                ke = min(kc + w_chunk, KT)
                st = st0 if kc < KH else st1
                lo = kc if kc < KH else kc - KH
                hi = lo + (ke - kc)
                if g < 3:
                    dma_eng = nc.sync if j % 2 == 0 else nc.scalar
                else:
                    dma_eng = (nc.sync, nc.scalar, nc.gpsimd)[j % 3]
                dma_eng.dma_start(out=st[:, lo:hi, :],
                                  in_=wTp[g, :, kc:ke, :])
                # beta stats on ACT (junk out into wbt, overwritten by Sign)
                nc.scalar.activation(
                    out=wbt[:, kc:ke, :].rearrange("p a b -> p (a b)"),
                    in_=st[:, lo:hi, :].rearrange("p a b -> p (a b)"),
                    func=ACTF.Abs, accum_out=sums_a[:, j:j + 1])
                if g < 2:
                    # alpha sums on ACT while DVE is quant-saturated
                    junk = junkp.tile([128, w_chunk, GS], FP8, tag="junk")
                    nc.scalar.activation(
                        out=junk.rearrange("p a b -> p (a b)"),
                        in_=st[:, lo:hi, :].rearrange("p a b -> p (a b)"),
                        func=ACTF.Copy,
                        accum_out=sums_w[:, j:j + 1])
                else:
                    nc.vector.tensor_reduce(
                        out=sums_w[:, kc:ke], in_=st[:, lo:hi, :],
                        axis=mybir.AxisListType.X, op=ALU.add)
            w_cols = n_ch if g < 2 else KT
            return (st0, st1), wbt, sums_w, sums_a, w_cols

        def chain_b(g, parts):
            with tc.high_priority():
                return _chain_b(g, parts)

        def _chain_b(g, parts):
            (st0, st1), wbt, sums_w, sums_a, w_cols = parts

            def stat_bcast(sums, cols, scale, tag):
                # cross-partition sum on PE, scale+total on ACT, PE broadcast
                cps = ps_small.tile([1, KT], F32, tag="fin")
                nc.tensor.matmul(cps[:, 0:cols], lhsT=ones_col,
                                 rhs=sums[:, 0:cols], start=True, stop=True)
                tot = smalls.tile([1, 1], F32, tag=tag + "t")
                jrow = smalls.tile([1, KT], FP8, tag="jrow")
                nc.scalar.activation(out=jrow[:, 0:cols], in_=cps[:, 0:cols],
                                     func=ACTF.Copy, scale=scale,
                                     accum_out=tot)
                bps = ps_small.tile([128, 1], F32, tag="finb")
                nc.tensor.matmul(bps, lhsT=ones_row, rhs=tot,
                                 start=True, stop=True)
                rep = smalls.tile([128, 1], F32, tag=tag)
                nc.scalar.activation(out=rep, in_=bps, func=ACTF.Copy)
                return rep

            narep = stat_bcast(sums_w, w_cols, -inv_gin, "narep")
            bqrep = stat_bcast(sums_a, n_ch, inv_gin, "bqrep")
            # binarize per half (frees each stage half for the next group)
            nc.scalar.activation(
                out=wbt[:, 0:KH, :].rearrange("p a b -> p (a b)"),
                in_=st0.rearrange("p a b -> p (a b)"),
                func=ACTF.Sign, bias=narep, scale=1.0)
            nc.scalar.activation(
                out=wbt[:, KH:KT, :].rearrange("p a b -> p (a b)"),
                in_=st1.rearrange("p a b -> p (a b)"),
                func=ACTF.Sign, bias=narep, scale=1.0)
            # bias slice, replicated across partitions
            brep = brepp.tile([128, GS], F32, tag="brep")
            bsrc = bias_d[g * GS:(g + 1) * GS]
            bsrc_b = bass.AP(tensor=bsrc.tensor, offset=bsrc.offset,
                             ap=[[0, 128]] + list(bsrc.ap))
            nc.gpsimd.dma_start(out=brep, in_=bsrc_b)
            return wbt, bqrep, brep

        # ---------------- emission ----------------
        pend = chain_a(0)
        cur = chain_b(0, pend)
        with tc.high_priority():
            emit_quant(0)
            emit_quant(1)
        pend = chain_a(1)
        for tb in range(2, TB):
            emit_quant(tb)

        for g in range(G):
            wbt, bqrep, brep = cur
            for t in range(TB):
                if t == 0 and pend is not None:
                    cur = chain_b(g + 1, pend)
                    pend = None
                if t == 2 and g + 2 < G:
                    pend = chain_a(g + 2)
                ps = psum.tile([128, GS], F32, tag="ps")
                n_mm = kp + (KT - kb0)
                mm = 0
                for k in range(kb0, KT):
                    nc.tensor.matmul(
                        ps, lhsT=qTb[:, k - kb0, t * 128:(t + 1) * 128],
                        rhs=wbt[:, k, :],
                        start=(mm == 0), stop=(mm == n_mm - 1),
                        skip_group_check=True)
                    mm += 1
                for p in range(kp):
                    nc.tensor.matmul(
                        ps, lhsT=qT8[:, 2 * p:2 * p + 2, t * 128:(t + 1) * 128],
                        rhs=wbt[:, 2 * p:2 * p + 2, :],
                        start=(mm == 0), stop=(mm == n_mm - 1),
                        perf_mode=DR, skip_group_check=True)
                    mm += 1
                ysb = ypool.tile([128, GS], F32, tag="ysb")
                svc = smalls.tile([128, 1], F32, tag="svc")
                nc.vector.tensor_scalar(
                    out=svc, in0=gcol_all[:, t:t + 1], scalar1=bqrep,
                    scalar2=None, op0=ALU.mult)
                nc.vector.scalar_tensor_tensor(
                    out=ysb, in0=ps, scalar=svc, in1=brep,
                    op0=ALU.mult, op1=ALU.add)
                y_eng = (nc.sync, nc.gpsimd)[t % 2]
                y_eng.dma_start(
                    out=y[t * 128:(t + 1) * 128, g * GS:(g + 1) * GS], in_=ysb)

    nc.finalize()
    return nc


_NC_CACHE = {}


def _get_nc():
    key = (KP,)
    if key not in _NC_CACHE:
        _NC_CACHE[key] = build_nc(KP)
    return _NC_CACHE[key]


def build_in_maps(inputs):
    x = np.asarray(inputs["x"])
    weight_fp = np.asarray(inputs["weight_fp"])
    bias = np.asarray(inputs["bias"])
    x_flat = x.reshape(-1, IN_F)
    wTp = np.ascontiguousarray(
        weight_fp.reshape(G, GS, KT, 128).transpose(0, 3, 2, 1))
    bias_c = np.ascontiguousarray(bias)
    maps = []
    for c in range(N_CORES):
        xs = x_flat[c * T:(c + 1) * T]
        # xTb[tb, p, kt, c] = x[tb*128 + c, kt*128 + p]
        xtb = np.ascontiguousarray(
            xs.reshape(TB, 128, KT, 128).transpose(0, 3, 2, 1))
        maps.append({"xTb": xtb, "wTp": wTp, "bias": bias_c})
    return maps


def kernel(x, weight_fp, bias, _want_results=False, **_kw):
    x = np.asarray(x)
    orig_shape = x.shape
    in_maps = build_in_maps({"x": x, "weight_fp": weight_fp, "bias": bias})
    nc = _get_nc()
    res = run_bass_kernel_spmd(nc, in_maps, core_ids=list(range(N_CORES)))
    yv = np.concatenate([r["y"] for r in res.results], axis=0)
    yv = yv.reshape(orig_shape[:-1] + (OUT_F,)).astype(np.float32)
    if _want_results:
        return yv, res
    return yv

